# revision 13
# baseline (speedup 1.0000x reference)
import sys

for p in ("/opt/trn_rl_repo",):
    if p not in sys.path:
        sys.path.insert(0, p)

import numpy as np
import ml_dtypes

import concourse.bass as bass
from concourse import bacc
import concourse.mybir as mybir
import concourse.tile as tile
from concourse.bass import ds, ts
from concourse.bass_utils import run_bass_kernel_spmd

BF16 = ml_dtypes.bfloat16

B, N, DIM, NH = 256, 196, 256, 8
HD = DIM // NH  # 32
G = 14
NCORES = 8
BLOC = B // NCORES  # 32

# all stationary operands are 128 columns wide (FWL-eligible, bf16):
#  - S matmul m-chunks (output partitions): rows 0:128 and 68:196 (overlap 68:128)
#  - PV contraction chunks (base-0 aligned slices of the S tiles): 0:68 and 68:196
#  - PV lhsT n-chunks / transpose n-chunks / proj n-chunks: 0:128 and 68:196
MCH = ((0, 128), (68, 128))       # S output m-chunks (m0, size)
NCH = ((0, 128), (68, 128))       # n-chunks (n0, 128) with overlap


def _relative_position_index(g: int) -> np.ndarray:
    coords = np.stack(np.meshgrid(np.arange(g), np.arange(g), indexing="ij"))
    cf = coords.reshape(2, -1)
    rel = cf[:, :, None] - cf[:, None, :]
    rel = rel.transpose(1, 2, 0).astype(np.int64)
    rel[..., 0] += g - 1
    rel[..., 1] += g - 1
    rel[..., 0] *= 2 * g - 1
    return rel.sum(-1)


def _bias_coords(g: int) -> np.ndarray:
    p = np.arange(1 - g, g)
    biases = np.stack(np.meshgrid(p, p, indexing="ij"))
    return biases.reshape(2, -1).T.astype(np.float32)


_CACHED = {}


def _build_bass():
    if "nc" in _CACHED:
        return _CACHED["nc"]
    f32 = mybir.dt.float32
    bf16 = mybir.dt.bfloat16

    nc = bacc.Bacc("TRN2", target_bir_lowering=False)
    qt_d = nc.dram_tensor("qt", [BLOC, 32, 8, 196], bf16, kind="ExternalInput")
    kt_d = nc.dram_tensor("kt", [BLOC, 32, 8, 196], bf16, kind="ExternalInput")
    vx_d = nc.dram_tensor("vx", [BLOC, 196, 8, 33], bf16, kind="ExternalInput")
    erpb_d = nc.dram_tensor("erpb", [128, 4, 2, 2, 196], bf16, kind="ExternalInput")
    w_d = nc.dram_tensor("w", [128, 2, 256], bf16, kind="ExternalInput")
    id_d = nc.dram_tensor("ident", [128, 128], bf16, kind="ExternalInput")
    id2_d = nc.dram_tensor("ident2", [128, 68], bf16, kind="ExternalInput")
    out_d = nc.dram_tensor("out", [BLOC, 196, 256], bf16, kind="ExternalOutput")

    from contextlib import ExitStack

    with tile.TileContext(nc) as tc, ExitStack() as es:
        const = es.enter_context(tc.tile_pool(name="const", bufs=1))
        io = es.enter_context(tc.tile_pool(name="io", bufs=4))
        work = es.enter_context(tc.tile_pool(name="work", bufs=4))
        pstp = es.enter_context(tc.tile_pool(name="pstp", bufs=8))
        s_pool = es.enter_context(tc.tile_pool(name="s_ps", bufs=2, space="PSUM"))
        x_pool = es.enter_context(tc.tile_pool(name="x_ps", bufs=1, space="PSUM"))
        t_pool = es.enter_context(tc.tile_pool(name="t_ps", bufs=1, space="PSUM"))
        o_pool = es.enter_context(tc.tile_pool(name="o_ps", bufs=1, space="PSUM"))

        erpb_sb = const.tile([128, 4, 2, 2, 196], bf16)
        w_sb = const.tile([128, 2, 256], bf16)
        id_sb = const.tile([128, 128], bf16)
        id2_sb = const.tile([128, 68], bf16)

        for b in range(BLOC):
            qt_sb = io.tile([32, 8, 196], bf16, tag="qt")
            nc.sync.dma_start(qt_sb[:], qt_d[b])
            kt_sb = io.tile([32, 8, 196], bf16, tag="kt")
            nc.sync.dma_start(kt_sb[:], kt_d[b])
            vx0_sb = io.tile([68, 8, 33], bf16, tag="vx0")
            nc.sync.dma_start(vx0_sb[:], vx_d[b, 0:68])
            vx1_sb = io.tile([128, 8, 33], bf16, tag="vx1")
            nc.sync.dma_start(vx1_sb[:], vx_d[b, 68:196])
            if b == 0:
                # const loads issued after the first window's inputs so the
                # first S matmuls are not stuck behind 230KB of constants
                nc.sync.dma_start(erpb_sb[:], erpb_d[:])
                nc.sync.dma_start(w_sb[:], w_d[:])
                nc.sync.dma_start(id_sb[:], id_d[:])
                nc.sync.dma_start(id2_sb[:], id2_d[:])

            # --- S = K^T Q -> exp -> * erpb.  Tile rows: chunk c covers
            # m = MCH[c][0] + p.  Bank-padded psum tiles (no bank sharing).
            pst_tiles = []
            for g in range(4):
                sps = s_pool.tile([128, 2, 2, 256], f32, tag="s")
                for a in range(2):
                    h = 2 * g + a
                    for c, (m0, msz) in enumerate(MCH):
                        nc.tensor.matmul(
                            sps[:, a, c, 0:196],
                            lhsT=kt_sb[:, h, ds(m0, msz)],
                            rhs=qt_sb[:, h, :],
                            start=True,
                            stop=True,
                        )
                est = work.tile([128, 2, 2, 196], bf16, tag="est")
                nc.scalar.activation(
                    est[:], sps[:, :, :, 0:196], mybir.ActivationFunctionType.Exp
                )
                pst = pstp.tile([128, 2, 2, 196], bf16, tag="pst", name=f"pst{g}")
                nc.vector.tensor_mul(out=pst[:], in0=est[:], in1=erpb_sb[:, g])
                pst_tiles.append(pst)

            # --- PV: x[n-chunk, h, d(+denom)].  Contraction split base-0:
            #   j=0: m 0..67   = pst[c=0] rows 0:68,  vx rows 0:68
            #   j=1: m 68..195 = pst[c=1] rows 0:128, vx rows 68:196
            xps = [
                x_pool.tile([128, 8, 64], f32, tag=f"x{ci}", name=f"x{ci}")
                for ci in range(2)
            ]
            for h in range(8):
                g, a = h // 2, h % 2
                for ci, (n0, nsz) in enumerate(NCH):
                    nc.tensor.matmul(
                        xps[ci][:, h, 0:33],
                        lhsT=pst_tiles[g][0:68, a, 0, ds(n0, nsz)],
                        rhs=vx0_sb[:, h, :],
                        start=True,
                        stop=False,
                    )
                    nc.tensor.matmul(
                        xps[ci][:, h, 0:33],
                        lhsT=pst_tiles[g][0:128, a, 1, ds(n0, nsz)],
                        rhs=vx1_sb[:, h, :],
                        start=False,
                        stop=True,
                    )

            # --- normalize
            xn = []
            for ci in range(2):
                rc = work.tile([128, 8], f32, tag=f"rc{ci}")
                nc.vector.reciprocal(rc[:], xps[ci][:, :, 32])
                x_sb = work.tile([128, 8, 32], bf16, tag=f"xn{ci}")
                nc.vector.tensor_mul(
                    out=x_sb[:],
                    in0=xps[ci][:, :, 0:32],
                    in1=rc[:, :, None].to_broadcast([128, 8, 32]),
                )
                xn.append(x_sb)

            # --- transpose x [n, hd] -> xT [hd, n] via PE
            #   n 0..127  from xn[0][0:128]    (identity)
            #   n 128..195 from xn[1][60:128]  (shifted identity id2[60+r, r]=1)
            # chunk-1 uses a shifted identity (id2[60+r, r] = 1) so only the 68
            # non-overlapping output columns (n 128..195) are streamed.
            xT = t_pool.tile([128, 2, 512], bf16, tag="xT")
            for half in range(2):
                nc.tensor.transpose(
                    xT[:, half, 0:128],
                    xn[0][:, ds(4 * half, 4), :],
                    id_sb[:],
                )
                nc.tensor.transpose(
                    xT[:, half, ds(128, 68)],
                    xn[1][:, ds(4 * half, 4), :],
                    id2_sb[:],
                )
            xTsb = work.tile([128, 2, 196], bf16, tag="xTsb")
            nc.vector.tensor_copy(xTsb[:], xT[:, :, 0:196])

            # --- output projection, n-chunks {0:128, 68:196}
            po = o_pool.tile([128, 2, 256], f32, tag="po")
            for i, (n0, nsz) in enumerate(NCH):
                for half in range(2):
                    nc.tensor.matmul(
                        po[:, i, :],
                        lhsT=xTsb[:, half, ds(n0, nsz)],
                        rhs=w_sb[:, half],
                        start=(half == 0),
                        stop=(half == 1),
                    )
            o_sb = work.tile([128, 2, 256], bf16, tag="o")
            nc.scalar.copy(o_sb[:], po[:])
            nc.sync.dma_start(out_d[b, 0:68], o_sb[0:68, 0])
            nc.sync.dma_start(out_d[b, 68:196], o_sb[:, 1])

    nc.compile()
    _CACHED["nc"] = nc
    return nc


def _prep_host(q, k, v, dpb_w1, dpb_b1, dpb_w2, dpb_b2, proj_w, proj_b):
    scale = HD ** -0.5
    qs = (q.astype(np.float32) * scale).transpose(0, 2, 1).reshape(B, 8, 32, 196)
    qt = np.ascontiguousarray(qs.transpose(0, 2, 1, 3)).astype(BF16)
    ks = k.astype(np.float32).transpose(0, 2, 1).reshape(B, 8, 32, 196)
    kt = np.ascontiguousarray(ks.transpose(0, 2, 1, 3)).astype(BF16)
    # vx [B, m, h, 33]
    vr = v.astype(np.float32).reshape(B, 196, 8, 32)
    vx = np.concatenate([vr, np.ones(vr.shape[:-1] + (1,), np.float32)], axis=-1)
    vx = np.ascontiguousarray(vx).astype(BF16)
    # erpb [p, g, a, c, n] = exp(rpb[h=2g+a, n, m=MCH[c][0]+p])
    biases = _bias_coords(G)
    pos = np.maximum(biases @ dpb_w1 + dpb_b1, 0.0) @ dpb_w2 + dpb_b2
    idx = _relative_position_index(G).reshape(-1)
    rpb = pos[idx].reshape(N, N, NH).transpose(2, 0, 1)  # [h, n, m]
    er = np.exp(rpb).transpose(0, 2, 1)  # [h, m, n]
    erpb = np.empty((128, 4, 2, 2, 196), np.float32)
    for g in range(4):
        for a in range(2):
            h = 2 * g + a
            for c, (m0, msz) in enumerate(MCH):
                erpb[:, g, a, c, :] = er[h, m0:m0 + msz, :]
    erpb = erpb.astype(BF16)
    w = np.ascontiguousarray(proj_w.reshape(2, 128, 256).transpose(1, 0, 2)).astype(BF16)
    ident = np.eye(128, dtype=np.float32).astype(BF16)
    ident2 = np.zeros((128, 68), np.float32)
    for r in range(68):
        ident2[60 + r, r] = 1.0
    ident2 = ident2.astype(BF16)
    return qt, kt, vx, erpb, w, ident, ident2


def kernel(**inputs) -> np.ndarray:
    q = np.asarray(inputs["q"], np.float32)
    k = np.asarray(inputs["k"], np.float32)
    v = np.asarray(inputs["v"], np.float32)
    proj_b = np.asarray(inputs["proj_b"], np.float32)
    qt, kt, vx, erpb, w, ident, ident2 = _prep_host(
        q, k, v,
        np.asarray(inputs["dpb_w1"], np.float32),
        np.asarray(inputs["dpb_b1"], np.float32),
        np.asarray(inputs["dpb_w2"], np.float32),
        np.asarray(inputs["dpb_b2"], np.float32),
        np.asarray(inputs["proj_w"], np.float32),
        proj_b,
    )
    nc = _build_bass()
    in_maps = []
    for c in range(NCORES):
        sl = slice(c * BLOC, (c + 1) * BLOC)
        in_maps.append(
            {
                "qt": np.ascontiguousarray(qt[sl]),
                "kt": np.ascontiguousarray(kt[sl]),
                "vx": np.ascontiguousarray(vx[sl]),
                "erpb": erpb,
                "w": w,
                "ident": ident,
                "ident2": ident2,
            }
        )
    res = run_bass_kernel_spmd(
        nc, in_maps, core_ids=list(range(NCORES)), trace=bool(_CACHED.get("trace"))
    )
    _CACHED["last_results"] = res
    out = np.concatenate([r["out"] for r in res.results], axis=0).astype(np.float32)
    out = out + proj_b[None, None, :]
    return out


if __name__ == "__main__":
    rng = np.random.default_rng(0)
    ins = {
        "q": rng.standard_normal((B, N, DIM), dtype=np.float32),
        "k": rng.standard_normal((B, N, DIM), dtype=np.float32),
        "v": rng.standard_normal((B, N, DIM), dtype=np.float32),
        "dpb_w1": rng.standard_normal((2, 64), dtype=np.float32) * 0.1,
        "dpb_b1": np.zeros(64, np.float32),
        "dpb_w2": rng.standard_normal((64, 8), dtype=np.float32) * 0.1,
        "dpb_b2": np.zeros(8, np.float32),
        "proj_w": rng.standard_normal((256, 256), dtype=np.float32) * (256 ** -0.5),
        "proj_b": np.zeros(256, np.float32),
        "group_size": 14,
    }
    o = kernel(**ins)
    print(o.shape, o.dtype)


# revision 14
# speedup vs baseline: 1.0025x; 1.0025x over previous
import sys

for p in ("/opt/trn_rl_repo",):
    if p not in sys.path:
        sys.path.insert(0, p)

import numpy as np
import ml_dtypes

import concourse.bass as bass
from concourse import bacc
import concourse.mybir as mybir
import concourse.tile as tile
from concourse.bass import ds, ts
from concourse.bass_utils import run_bass_kernel_spmd

BF16 = ml_dtypes.bfloat16

B, N, DIM, NH = 256, 196, 256, 8
HD = DIM // NH  # 32
G = 14
NCORES = 8
BLOC = B // NCORES  # 32

# all stationary operands are 128 columns wide (FWL-eligible, bf16):
#  - S matmul m-chunks (output partitions): rows 0:128 and 68:196 (overlap 68:128)
#  - PV contraction chunks (base-0 aligned slices of the S tiles): 0:68 and 68:196
#  - PV lhsT n-chunks / transpose n-chunks / proj n-chunks: 0:128 and 68:196
MCH = ((0, 128), (68, 128))       # S output m-chunks (m0, size)
NCH = ((0, 128), (68, 128))       # n-chunks (n0, 128) with overlap


def _relative_position_index(g: int) -> np.ndarray:
    coords = np.stack(np.meshgrid(np.arange(g), np.arange(g), indexing="ij"))
    cf = coords.reshape(2, -1)
    rel = cf[:, :, None] - cf[:, None, :]
    rel = rel.transpose(1, 2, 0).astype(np.int64)
    rel[..., 0] += g - 1
    rel[..., 1] += g - 1
    rel[..., 0] *= 2 * g - 1
    return rel.sum(-1)


def _bias_coords(g: int) -> np.ndarray:
    p = np.arange(1 - g, g)
    biases = np.stack(np.meshgrid(p, p, indexing="ij"))
    return biases.reshape(2, -1).T.astype(np.float32)


_CACHED = {}


def _build_bass():
    if "nc" in _CACHED:
        return _CACHED["nc"]
    f32 = mybir.dt.float32
    bf16 = mybir.dt.bfloat16

    nc = bacc.Bacc("TRN2", target_bir_lowering=False)
    qt_d = nc.dram_tensor("qt", [BLOC, 32, 8, 196], bf16, kind="ExternalInput")
    kt_d = nc.dram_tensor("kt", [BLOC, 32, 8, 196], bf16, kind="ExternalInput")
    vx_d = nc.dram_tensor("vx", [BLOC, 196, 8, 33], bf16, kind="ExternalInput")
    erpb_d = nc.dram_tensor("erpb", [128, 4, 2, 2, 196], bf16, kind="ExternalInput")
    w_d = nc.dram_tensor("w", [128, 2, 256], bf16, kind="ExternalInput")
    id_d = nc.dram_tensor("ident", [128, 128], bf16, kind="ExternalInput")
    id2_d = nc.dram_tensor("ident2", [128, 68], bf16, kind="ExternalInput")
    out_d = nc.dram_tensor("out", [BLOC, 196, 256], bf16, kind="ExternalOutput")

    from contextlib import ExitStack

    with tile.TileContext(nc) as tc, ExitStack() as es:
        const = es.enter_context(tc.tile_pool(name="const", bufs=1))
        io = es.enter_context(tc.tile_pool(name="io", bufs=3))
        work = es.enter_context(tc.tile_pool(name="work", bufs=3))
        pstp = es.enter_context(tc.tile_pool(name="pstp", bufs=8))
        s_pool = es.enter_context(tc.tile_pool(name="s_ps", bufs=2, space="PSUM"))
        x_pool = es.enter_context(tc.tile_pool(name="x_ps", bufs=1, space="PSUM"))
        t_pool = es.enter_context(tc.tile_pool(name="t_ps", bufs=1, space="PSUM"))
        o_pool = es.enter_context(tc.tile_pool(name="o_ps", bufs=1, space="PSUM"))

        erpb_sb = const.tile([128, 4, 2, 2, 196], bf16)
        w_sb = const.tile([128, 2, 256], bf16)
        id_sb = const.tile([128, 128], bf16)
        id2_sb = const.tile([128, 68], bf16)

        for b in range(BLOC):
            qt_sb = io.tile([32, 8, 196], bf16, tag="qt")
            nc.sync.dma_start(qt_sb[:], qt_d[b])
            kt_sb = io.tile([32, 8, 196], bf16, tag="kt")
            nc.sync.dma_start(kt_sb[:], kt_d[b])
            vx0_sb = io.tile([68, 8, 33], bf16, tag="vx0")
            nc.sync.dma_start(vx0_sb[:], vx_d[b, 0:68])
            vx1_sb = io.tile([128, 8, 33], bf16, tag="vx1")
            nc.sync.dma_start(vx1_sb[:], vx_d[b, 68:196])
            if b == 0:
                # const loads issued after the first window's inputs so the
                # first S matmuls are not stuck behind 230KB of constants
                nc.sync.dma_start(erpb_sb[:], erpb_d[:])
                nc.sync.dma_start(w_sb[:], w_d[:])
                nc.sync.dma_start(id_sb[:], id_d[:])
                nc.sync.dma_start(id2_sb[:], id2_d[:])

            # --- S = K^T Q -> exp -> * erpb.  Tile rows: chunk c covers
            # m = MCH[c][0] + p.  Bank-padded psum tiles (no bank sharing).
            pst_tiles = []
            for g in range(4):
                sps = s_pool.tile([128, 2, 2, 256], f32, tag="s")
                for a in range(2):
                    h = 2 * g + a
                    for c, (m0, msz) in enumerate(MCH):
                        nc.tensor.matmul(
                            sps[:, a, c, 0:196],
                            lhsT=kt_sb[:, h, ds(m0, msz)],
                            rhs=qt_sb[:, h, :],
                            start=True,
                            stop=True,
                        )
                est = work.tile([128, 2, 2, 196], bf16, tag="est")
                nc.scalar.activation(
                    est[:], sps[:, :, :, 0:196], mybir.ActivationFunctionType.Exp
                )
                pst = pstp.tile([128, 2, 2, 196], bf16, tag="pst", name=f"pst{g}")
                nc.vector.tensor_mul(out=pst[:], in0=est[:], in1=erpb_sb[:, g])
                pst_tiles.append(pst)

            # --- PV: x[n-chunk, h, d(+denom)].  Contraction split base-0:
            #   j=0: m 0..67   = pst[c=0] rows 0:68,  vx rows 0:68
            #   j=1: m 68..195 = pst[c=1] rows 0:128, vx rows 68:196
            xps = [
                x_pool.tile([128, 8, 64], f32, tag=f"x{ci}", name=f"x{ci}")
                for ci in range(2)
            ]
            for h in range(8):
                g, a = h // 2, h % 2
                for ci, (n0, nsz) in enumerate(NCH):
                    nc.tensor.matmul(
                        xps[ci][:, h, 0:33],
                        lhsT=pst_tiles[g][0:68, a, 0, ds(n0, nsz)],
                        rhs=vx0_sb[:, h, :],
                        start=True,
                        stop=False,
                    )
                    nc.tensor.matmul(
                        xps[ci][:, h, 0:33],
                        lhsT=pst_tiles[g][0:128, a, 1, ds(n0, nsz)],
                        rhs=vx1_sb[:, h, :],
                        start=False,
                        stop=True,
                    )

            # --- normalize
            xn = []
            for ci in range(2):
                rc = work.tile([128, 8], f32, tag=f"rc{ci}")
                nc.vector.reciprocal(rc[:], xps[ci][:, :, 32])
                x_sb = work.tile([128, 8, 32], bf16, tag=f"xn{ci}")
                nc.vector.tensor_mul(
                    out=x_sb[:],
                    in0=xps[ci][:, :, 0:32],
                    in1=rc[:, :, None].to_broadcast([128, 8, 32]),
                )
                xn.append(x_sb)

            # --- transpose x [n, hd] -> xT [hd, n] via PE
            #   n 0..127  from xn[0][0:128]    (identity)
            #   n 128..195 from xn[1][60:128]  (shifted identity id2[60+r, r]=1)
            # chunk-1 uses a shifted identity (id2[60+r, r] = 1) so only the 68
            # non-overlapping output columns (n 128..195) are streamed.
            xT = t_pool.tile([128, 2, 512], bf16, tag="xT")
            for half in range(2):
                nc.tensor.transpose(
                    xT[:, half, 0:128],
                    xn[0][:, ds(4 * half, 4), :],
                    id_sb[:],
                )
                nc.tensor.transpose(
                    xT[:, half, ds(128, 68)],
                    xn[1][:, ds(4 * half, 4), :],
                    id2_sb[:],
                )
            xTsb = work.tile([128, 2, 196], bf16, tag="xTsb")
            nc.vector.tensor_copy(xTsb[:], xT[:, :, 0:196])

            # --- output projection, n-chunks {0:128, 68:196}
            po = o_pool.tile([128, 2, 256], f32, tag="po")
            for i, (n0, nsz) in enumerate(NCH):
                for half in range(2):
                    nc.tensor.matmul(
                        po[:, i, :],
                        lhsT=xTsb[:, half, ds(n0, nsz)],
                        rhs=w_sb[:, half],
                        start=(half == 0),
                        stop=(half == 1),
                    )
            o_sb = work.tile([128, 2, 256], bf16, tag="o")
            nc.scalar.copy(o_sb[:], po[:])
            nc.sync.dma_start(out_d[b, 0:68], o_sb[0:68, 0])
            nc.sync.dma_start(out_d[b, 68:196], o_sb[:, 1])

    nc.compile()
    _CACHED["nc"] = nc
    return nc


def _prep_host(q, k, v, dpb_w1, dpb_b1, dpb_w2, dpb_b2, proj_w, proj_b):
    scale = HD ** -0.5
    qs = (q.astype(np.float32) * scale).transpose(0, 2, 1).reshape(B, 8, 32, 196)
    qt = np.ascontiguousarray(qs.transpose(0, 2, 1, 3)).astype(BF16)
    ks = k.astype(np.float32).transpose(0, 2, 1).reshape(B, 8, 32, 196)
    kt = np.ascontiguousarray(ks.transpose(0, 2, 1, 3)).astype(BF16)
    # vx [B, m, h, 33]
    vr = v.astype(np.float32).reshape(B, 196, 8, 32)
    vx = np.concatenate([vr, np.ones(vr.shape[:-1] + (1,), np.float32)], axis=-1)
    vx = np.ascontiguousarray(vx).astype(BF16)
    # erpb [p, g, a, c, n] = exp(rpb[h=2g+a, n, m=MCH[c][0]+p])
    biases = _bias_coords(G)
    pos = np.maximum(biases @ dpb_w1 + dpb_b1, 0.0) @ dpb_w2 + dpb_b2
    idx = _relative_position_index(G).reshape(-1)
    rpb = pos[idx].reshape(N, N, NH).transpose(2, 0, 1)  # [h, n, m]
    er = np.exp(rpb).transpose(0, 2, 1)  # [h, m, n]
    erpb = np.empty((128, 4, 2, 2, 196), np.float32)
    for g in range(4):
        for a in range(2):
            h = 2 * g + a
            for c, (m0, msz) in enumerate(MCH):
                erpb[:, g, a, c, :] = er[h, m0:m0 + msz, :]
    erpb = erpb.astype(BF16)
    w = np.ascontiguousarray(proj_w.reshape(2, 128, 256).transpose(1, 0, 2)).astype(BF16)
    ident = np.eye(128, dtype=np.float32).astype(BF16)
    ident2 = np.zeros((128, 68), np.float32)
    for r in range(68):
        ident2[60 + r, r] = 1.0
    ident2 = ident2.astype(BF16)
    return qt, kt, vx, erpb, w, ident, ident2


def kernel(**inputs) -> np.ndarray:
    q = np.asarray(inputs["q"], np.float32)
    k = np.asarray(inputs["k"], np.float32)
    v = np.asarray(inputs["v"], np.float32)
    proj_b = np.asarray(inputs["proj_b"], np.float32)
    qt, kt, vx, erpb, w, ident, ident2 = _prep_host(
        q, k, v,
        np.asarray(inputs["dpb_w1"], np.float32),
        np.asarray(inputs["dpb_b1"], np.float32),
        np.asarray(inputs["dpb_w2"], np.float32),
        np.asarray(inputs["dpb_b2"], np.float32),
        np.asarray(inputs["proj_w"], np.float32),
        proj_b,
    )
    nc = _build_bass()
    in_maps = []
    for c in range(NCORES):
        sl = slice(c * BLOC, (c + 1) * BLOC)
        in_maps.append(
            {
                "qt": np.ascontiguousarray(qt[sl]),
                "kt": np.ascontiguousarray(kt[sl]),
                "vx": np.ascontiguousarray(vx[sl]),
                "erpb": erpb,
                "w": w,
                "ident": ident,
                "ident2": ident2,
            }
        )
    res = run_bass_kernel_spmd(
        nc, in_maps, core_ids=list(range(NCORES)), trace=bool(_CACHED.get("trace"))
    )
    _CACHED["last_results"] = res
    out = np.concatenate([r["out"] for r in res.results], axis=0).astype(np.float32)
    out = out + proj_b[None, None, :]
    return out


if __name__ == "__main__":
    rng = np.random.default_rng(0)
    ins = {
        "q": rng.standard_normal((B, N, DIM), dtype=np.float32),
        "k": rng.standard_normal((B, N, DIM), dtype=np.float32),
        "v": rng.standard_normal((B, N, DIM), dtype=np.float32),
        "dpb_w1": rng.standard_normal((2, 64), dtype=np.float32) * 0.1,
        "dpb_b1": np.zeros(64, np.float32),
        "dpb_w2": rng.standard_normal((64, 8), dtype=np.float32) * 0.1,
        "dpb_b2": np.zeros(8, np.float32),
        "proj_w": rng.standard_normal((256, 256), dtype=np.float32) * (256 ** -0.5),
        "proj_b": np.zeros(256, np.float32),
        "group_size": 14,
    }
    o = kernel(**ins)
    print(o.shape, o.dtype)
